# revision 35
# baseline (speedup 1.0000x reference)
"""Trainium2 Bass kernel for nn_EquivariantUpdateLayer (GNN message passing).

Edge-parallel across 8 NeuronCores, destination-sorted edge tiles so the
per-node aggregation is local to each core (per sharding hint).

v2 design (vs the HBM-gather baseline):
- The whole h table lives in SBUF as bf16 ([128, 393*128] token layout,
  row n -> partition n%128, rank n//128, per-core ROTATED by the core's
  window base so hi tokens start at rank 1).  Gathers are SBUF->SBUF
  dma_gather with prepare_only + trigger_dma so the Pool engine only pays
  descriptor generation, not the transfer.
- 2 gathers per 8-tile batch: A = [hi tokens | hj-lo tokens] (full view),
  B = hj-hi tokens (view offset 255 ranks).  int16 token limit handled by
  the rank-0/rank-392 zero rows as dummy targets; hjT = A_hj + B on DVE.
- LayerNorm: mean correction folded into W2 (W2c = W2g - 1*(1^T W2g)/H),
  so only rstd is needed per edge.  Stats matmuls emit mu / E[s1^2]
  directly (selector value 1/H); var = E[s1^2] - mu^2 via a negated
  identity matmul accumulated into the stats PSUM; rstd = ACT Rsqrt.
  rstd broadcast to [128, e] via a ones-column matmul into PSUM.
- Scale tail: vec = (scp + b4) * dx in one scalar_tensor_tensor; pane
  one-hot built in one batched is_equal over [128, TPC, 128].
- Scatter: per-tile [128,3] panes staged, dma_scatter_add (prep+trigger)
  into a DRAM agg table on disjoint rows; tail adds x.
Host prep does only index manipulation, dtype casts, permutation/padding.
"""
import hashlib
import numpy as np
import ml_dtypes

bf16 = ml_dtypes.bfloat16
f32 = np.float32

NCORES = 8
CHUNK = 128
TPC = 4
TILE = CHUNK * TPC
G = 16             # tiles per LN-stats group
WBITS = 7
WSZ = 128
H = 128
EPS = 1e-5
GB = 8             # tiles per gather/scatter batch
TRACE = False      # set True to capture an NTFF profile (exec_time_ns)
TRACE_DIR = None   # optional fixed dir for NTFF/perfetto artifacts
ACT_FN = "Silu"    # sim_test swaps to Sigmoid (CoreSim lacks Silu)
STAGE = 5          # debug: 1=z1/silu1 2=+stats/LN 3=+z2..scale 4=+scatter 5=full

N_PAD = 50048          # 391*128 (h rows padded)
NRANKS = 391
TOKC = 393 * 128       # table elems/partition incl zero ranks 0 and 392
HIVIEW = 255 * 128     # elem offset of the hj-hi view (rank 255)
LO_MAX = 254 * 128     # rot < LO_MAX uses the lo view (idx = rot+128 < 32640)


# ---------------------------------------------------------------- host prep --

def _pack_tiles(ei_sorted):
    """Pack whole destination nodes into tiles of <=512 edges spanning <128 nodes.
    Returns list of tiles: (edge_start, edge_end, base_node)."""
    nodes, counts = np.unique(ei_sorted, return_counts=True)
    nodes = nodes.tolist()
    counts = counts.tolist()
    tiles = []
    cur_s = 0
    cur_e = 0
    cur_base = -1
    pos = 0
    for node, deg in zip(nodes, counts):
        assert deg <= TILE, f"node degree {deg} > {TILE} unsupported"
        fits = (cur_base >= 0 and (cur_e - cur_s) + deg <= TILE
                and node - cur_base < WSZ)
        if not fits:
            if cur_base >= 0:
                tiles.append((cur_s, cur_e, cur_base))
            cur_s = pos
            cur_e = pos
            cur_base = node
        cur_e += deg
        pos += deg
    if cur_base >= 0:
        tiles.append((cur_s, cur_e, cur_base))
    return tiles


def _wrap16(idx):
    w = idx.reshape(-1, 16).T.astype(np.int16)
    return np.ascontiguousarray(np.tile(w, (8, 1)))


def _prepare(h, x, e, dx, d2):
    N = h.shape[0]
    assert N <= N_PAD
    order = np.argsort(e[0], kind="stable")
    ei = e[0][order].astype(np.int64)
    ej = e[1][order].astype(np.int64)
    dxs = np.asarray(dx, f32)[order]
    d2s = np.asarray(d2, f32)[order][:, 0]

    tiles = _pack_tiles(ei)
    ntiles_tot = len(tiles)
    NT = -(-ntiles_tot // NCORES)
    ngroups = -(-NT // G)
    gsizes = tuple(min(G, NT - g * G) for g in range(ngroups))
    NB = -(-NT // GB)

    cores = []
    for c in range(NCORES):
        lo = min(c * NT, ntiles_tot)
        hi = min(lo + NT, ntiles_tot)
        ct = tiles[lo:hi]
        if ct:
            wfirst = ct[0][2] >> WBITS
            wlast = (ct[-1][2] + WSZ - 1) >> WBITS
        else:
            wfirst, wlast = 0, 0
        cores.append({"tiles": ct, "wfirst": wfirst,
                      "nwin": max(wlast - wfirst + 1, 1)})
    nwin_cap = max(cr["nwin"] for cr in cores)
    nwin_cap = -(-nwin_cap // 8) * 8
    R = nwin_cap * WSZ + WSZ  # + dump zone; dump row = R-1
    assert R <= 32767
    assert nwin_cap * WSZ + 256 < LO_MAX, "window exceeds lo-view token range"

    C = NT * TILE
    CB = NB * GB * TILE  # batch-padded edge capacity
    h_pad = np.zeros((N_PAD, H), f32)
    h_pad[:N] = np.asarray(h, f32)
    h_pad_b = h_pad.astype(bf16)

    # ownership boundaries for the host combine
    bnds = []
    prev = 0
    for cc in range(NCORES):
        if cores[cc]["tiles"]:
            prev = cores[cc]["tiles"][-1][2] + WSZ
        bnds.append(prev)

    data = []
    for c in range(NCORES):
        cr = cores[c]
        base0 = cr["wfirst"] << WBITS
        ei_f = np.full(CB, base0, np.int64)
        ej_f = np.zeros(CB, np.int64)
        dx_f = np.zeros((CB, 3), f32)
        d2_f = np.zeros(CB, f32)
        rel_f = np.zeros(CB, f32)
        tbase = np.full(NT, base0, np.int64)
        tspan = np.zeros(NT, np.int64)
        for k, (s, t, tb) in enumerate(cr["tiles"]):
            n = t - s
            sl = slice(k * TILE, k * TILE + n)
            ei_f[sl] = ei[s:t]
            ej_f[sl] = ej[s:t]
            dx_f[sl] = dxs[s:t]
            d2_f[sl] = d2s[s:t]
            rel_f[sl] = (ei[s:t] - tb).astype(f32)
            ei_f[k * TILE + n:(k + 1) * TILE] = tb
            tbase[k] = tb
            tspan[k] = int(ei[t - 1] - tb + 1) if n else 0

        # gather tokens (rotated table: rank 1+k holds rows base0+128k..)
        hi_tok = ei_f - base0 + 128
        rot = (ej_f - base0) % N_PAD
        in_lo = rot < LO_MAX
        # lo view: idx = rot+128 in [128, 32640); dummy = rank 0 (zeros)
        # hi view (offset 255 ranks): abs rank 255 + t//128 = 1 + rot//128
        #   -> t = rot - 254*128 = rot - 32512; dummy = view rank 137 (zeros)
        tokA_hj = np.where(in_lo, rot + 128, ej_f % 128)
        tokB = np.where(in_lo, 137 * 128 + (ej_f % 128), rot - 32512)
        assert tokB.max() < 138 * 128 and tokB.min() >= 0
        assert tokA_hj.max() < 32640 and hi_tok.max() < 32640

        # per-batch idx arrays: A = [hi | hj-lo] (8192), B = hj-hi (4096)
        idxA = np.zeros((NB, 2 * GB * TILE), np.int64)
        idxB = np.zeros((NB, GB * TILE), np.int64)
        for b in range(NB):
            sl = slice(b * GB * TILE, (b + 1) * GB * TILE)
            idxA[b, :GB * TILE] = hi_tok[sl]
            idxA[b, GB * TILE:] = tokA_hj[sl]
            idxB[b] = tokB[sl]
        idxA_w = _wrap16(idxA.reshape(-1))
        idxB_w = _wrap16(idxB.reshape(-1))

        # host pane-placement indices: node row <- (pane partition p, tile t)
        rows_l, p_l, t_l = [], [], []
        for k in range(NT):
            sp = int(tspan[k])
            if sp > 0:
                rows_l.append(tbase[k] + np.arange(sp))
                p_l.append(np.arange(sp))
                t_l.append(np.full(sp, k))
        place_rows = np.concatenate(rows_l) if rows_l else np.zeros(0, np.int64)
        place_p = np.concatenate(p_l) if p_l else np.zeros(0, np.int64)
        place_t = np.concatenate(t_l) if t_l else np.zeros(0, np.int64)

        rel_p = rel_f.reshape(NB * GB, TPC, CHUNK).transpose(0, 2, 1)
        rel_packed = rel_p.transpose(1, 0, 2).reshape(CHUNK, NB * GB * TPC)
        dx_p = dx_f.reshape(NB * GB, TPC, CHUNK, 3).transpose(0, 2, 1, 3)
        dx_packed = dx_p.transpose(1, 0, 2, 3).reshape(CHUNK, NB * GB * TPC * 3)

        # rotated token table [128, TOKC]: rank0/rank392 zeros
        q = np.arange(NRANKS)
        p = np.arange(128)
        rows_idx = (base0 + q[None, :] * 128 + p[:, None]) % N_PAD  # [128, 391]
        tbl = h_pad_b[rows_idx]                                     # [128,391,H]
        h_tok = np.zeros((128, TOKC), bf16)
        h_tok[:, 128:128 + NRANKS * 128] = tbl.reshape(128, NRANKS * 128)

        data.append({
            "h_tok": np.ascontiguousarray(h_tok),
            "idxA": idxA_w, "idxB": idxB_w,
            "rel": np.ascontiguousarray(rel_packed.astype(bf16)),
            "dxp": np.ascontiguousarray(dx_packed, dtype=f32),
            "d2": np.ascontiguousarray(d2_f.astype(bf16)[None, :]),
            "place_rows": place_rows, "place_p": place_p, "place_t": place_t,
            "wfirst": cr["wfirst"],
        })

    shape_meta = {"C": C, "NT": NT, "NB": NB, "gsizes": gsizes,
                  "nwin_cap": nwin_cap, "R": R, "N": N}
    return data, shape_meta


# ------------------------------------------------------------- graph builder --

def _build(sm):
    import concourse.bass as bass
    import concourse.bacc as bacc
    import concourse.mybir as mybir
    import concourse.tile as tile

    NT, NB, nwin_cap, R = sm["NT"], sm["NB"], sm["nwin_cap"], sm["R"]
    gsizes = sm["gsizes"]
    AF = mybir.ActivationFunctionType
    AFACT = getattr(AF, ACT_FN)
    DT = mybir.dt
    ALU = mybir.AluOpType

    nc = bacc.Bacc("TRN2", num_devices=NCORES)

    def din(name, shape, dt):
        return nc.dram_tensor(name, shape, dt, kind="ExternalInput").ap()

    h_tok_d = din("h_tok", [128, TOKC], DT.bfloat16)
    idxA_d = din("idxA", [128, NB * 2 * GB * TILE // 16], DT.int16)
    idxB_d = din("idxB", [128, NB * GB * TILE // 16], DT.int16)
    rel_d = din("rel", [128, NB * GB * TPC], DT.bfloat16)
    dxp_d = din("dxp", [128, NB * GB * TPC * 3], DT.float32)
    d2_d = din("d2", [1, NB * GB * TILE], DT.bfloat16)
    sels_d = din("sels", [H, G * G], DT.bfloat16)
    selr_d = din("selr", [G, G * H], DT.bfloat16)
    negI_d = din("negI", [G, G], DT.bfloat16)
    W1_d = din("W1", [2 * H + 1, H], DT.float32)
    W2_d = din("W2", [H, H], DT.float32)
    W3_d = din("W3", [H, H], DT.float32)
    W4_d = din("W4", [H, 1], DT.float32)
    b1_d = din("b1", [H, 1], DT.float32)
    b2_d = din("b2", [H, 1], DT.float32)
    b3_d = din("b3", [H, 1], DT.float32)
    b4_d = din("b4", [H, 1], DT.float32)
    g1_d = din("g1", [H, 1], DT.float32)
    beta_d = din("beta1", [H, 1], DT.float32)
    out_d = nc.dram_tensor("out", [128, NT * 3], DT.float32,
                           kind="ExternalOutput").ap()

    with tile.TileContext(nc) as tc:
        _pools = []

        def _mkpool(**kw):
            p = tc.alloc_tile_pool(**kw)
            _pools.append(p)
            return p

        con = _mkpool(name="con", bufs=1)
        zps = _mkpool(name="zps", bufs=4, space="PSUM")
        sps = _mkpool(name="sps", bufs=1, space="PSUM")
        pps = _mkpool(name="pps", bufs=2, space="PSUM")
        gAp = _mkpool(name="gAp", bufs=2)
        gBp = _mkpool(name="gBp", bufs=2)
        gip = _mkpool(name="gip", bufs=2)
        s1p = _mkpool(name="s1p", bufs=G + 4)
        wkp = _mkpool(name="wkp", bufs=2)
        stp = _mkpool(name="stp", bufs=2)
        bsp = _mkpool(name="bsp", bufs=2)

        # ---- one-time constants ----
        def load_cast(dram_ap, shape, name):
            t_f = con.tile(shape, DT.float32, tag=f"{name}_f")
            nc.sync.dma_start(t_f[:], dram_ap)
            t_b = con.tile(shape, DT.bfloat16, tag=name)
            nc.vector.tensor_copy(t_b[:], t_f[:])
            return t_b

        W1a = load_cast(W1_d[0:H, :], [H, H], "W1a")
        W1b = load_cast(W1_d[H:2 * H, :], [H, H], "W1b")
        w1c = load_cast(W1_d[2 * H:2 * H + 1, :], [1, H], "w1c")
        W3b = load_cast(W3_d[:, :], [H, H], "W3b")
        W4b = load_cast(W4_d[:, :], [H, 1], "W4b")
        W2b = load_cast(W2_d[:, :], [H, H], "W2b")
        betab = load_cast(beta_d[:, :], [H, 1], "betab")

        def load_col(dram_ap, name):
            t = con.tile([H, 1], DT.float32, tag=name)
            nc.sync.dma_start(t[:], dram_ap)
            return t

        b1c = load_col(b1_d[:, :], "b1c")
        b2c = load_col(b2_d[:, :], "b2c")
        b3c = load_col(b3_d[:, :], "b3c")
        b4c = load_col(b4_d[:, :], "b4c")
        g1c = load_col(g1_d[:, :], "g1c")
        W2f = con.tile([H, H], DT.float32, tag="W2f")
        nc.sync.dma_start(W2f[:], W2_d[:, :])

        W2gf = con.tile([H, H], DT.float32, tag="W2gf")
        nc.vector.tensor_scalar_mul(W2gf[:], W2f[:], g1c[:])
        W2g = con.tile([H, H], DT.bfloat16, tag="W2g")
        nc.vector.tensor_copy(W2g[:], W2gf[:])

        onesc = con.tile([H, 1], DT.bfloat16, tag="onesc")
        nc.vector.memset(onesc[:], 1.0)
        ones1 = con.tile([1, 1], DT.bfloat16, tag="ones1")
        nc.vector.memset(ones1[:], 1.0)
        onesr = con.tile([1, H], DT.bfloat16, tag="onesr")
        nc.vector.memset(onesr[:], 1.0)

        # W2c = W2g - ones (x) (1^T W2g)/H   (folds -u*mu into W2)
        u_ps = zps.tile([1, H], DT.float32, space="PSUM", tag="z")
        nc.tensor.matmul(u_ps[:], lhsT=onesc[:], rhs=W2g[:], start=True, stop=True)
        u_row = con.tile([1, H], DT.bfloat16, tag="u_row")
        nc.vector.tensor_scalar_mul(u_row[:], u_ps[:], 1.0 / H)
        rk_ps = zps.tile([H, H], DT.float32, space="PSUM", tag="z")
        nc.tensor.matmul(rk_ps[:], lhsT=onesr[:], rhs=u_row[:], start=True, stop=True)
        W2c = con.tile([H, H], DT.bfloat16, tag="W2c")
        nc.vector.tensor_sub(W2c[:], W2gf[:], rk_ps[:])

        # b2p = W2^T beta + b2
        bb_ps = zps.tile([1, H], DT.float32, space="PSUM", tag="z")
        nc.tensor.matmul(bb_ps[:], lhsT=betab[:], rhs=W2b[:], start=True, stop=True)
        bb_row = con.tile([1, H], DT.bfloat16, tag="bb_row")
        nc.vector.tensor_copy(bb_row[:], bb_ps[:])
        bbT_ps = zps.tile([H, 1], DT.float32, space="PSUM", tag="z")
        nc.tensor.matmul(bbT_ps[:], lhsT=bb_row[:], rhs=ones1[:], start=True, stop=True)
        b2p = con.tile([H, 1], DT.float32, tag="b2p")
        nc.vector.tensor_add(b2p[:], bbT_ps[:], b2c[:])

        # iota4 [128, TPC, WSZ] bf16 (value = col within chunk, all partitions)
        iotai = con.tile([128, TPC, WSZ], DT.int32, tag="iotai")
        nc.gpsimd.iota(iotai[:], pattern=[[0, TPC], [1, WSZ]], base=0,
                       channel_multiplier=0)
        iota4 = con.tile([128, TPC, WSZ], DT.bfloat16, tag="iota4")
        nc.vector.tensor_copy(iota4[:], iotai[:])

        # stats selectors (value 1/H), row selectors for the rstd broadcast,
        # and a negated identity living at partitions 32:48 (so the var
        # matmul's operand bases match its tile_position row group).
        # Host-provided (per-row writes would need unaligned partition APs).
        sels_t = con.tile([H, G * G], DT.bfloat16, tag="sels_t")
        nc.sync.dma_start(sels_t[:], sels_d[:, :])
        sels = [sels_t[:, j * G:(j + 1) * G] for j in range(G)]
        selr_t = con.tile([G, G * H], DT.bfloat16, tag="selr_t")
        nc.sync.dma_start(selr_t[:], selr_d[:, :])
        selr = [selr_t[:, j * H:(j + 1) * H] for j in range(G)]
        negI = con.tile([32 + G, G], DT.bfloat16, tag="negI")
        nc.sync.dma_start(negI[32:32 + G, :], negI_d[:, :])
        epsc = con.tile([G, 1], DT.float32, tag="epsc")
        nc.vector.memset(epsc[:], EPS)

        # resident token table + per-tile pane accumulator
        ht = con.tile([128, TOKC], DT.bfloat16, tag="ht")
        nc.sync.dma_start(ht[:], h_tok_d[:, :])
        agg_sb = con.tile([128, NT, 3], DT.float32, tag="agg_sb")

        # ---- pipeline ----
        batch_bufs = {}
        s1_tiles = {}
        stats_cur = [None]
        bstream = {}

        def gather_batch(b):
            iA = gip.tile([128, 2 * GB * TILE // 16], DT.int16, tag="iA")
            nc.sync.dma_start(iA[:], idxA_d[:, b * (2 * GB * TILE // 16):
                                            (b + 1) * (2 * GB * TILE // 16)])
            iB = gip.tile([128, GB * TILE // 16], DT.int16, tag="iB")
            nc.sync.dma_start(iB[:], idxB_d[:, b * (GB * TILE // 16):
                                            (b + 1) * (GB * TILE // 16)])
            gA = gAp.tile([128, 1, 2 * GB * TILE], DT.bfloat16, tag="gA")
            gB = gBp.tile([128, 1, GB * TILE], DT.bfloat16, tag="gB")
            nc.gpsimd.dma_gather(
                out_ap=gA[:, :, :], in_ap=ht[:, :], idxs_ap=iA[:, :],
                num_idxs=2 * GB * TILE, num_idxs_reg=2 * GB * TILE,
                elem_size=H, transpose=True, single_packet=False,
                sbuf_tokens_per_rank=128, sbuf_free_dim_per_rank=256)
            nc.gpsimd.dma_gather(
                out_ap=gB[:, :, :], in_ap=ht[:, HIVIEW:], idxs_ap=iB[:, :],
                num_idxs=GB * TILE, num_idxs_reg=GB * TILE,
                elem_size=H, transpose=True, single_packet=False,
                sbuf_tokens_per_rank=128, sbuf_free_dim_per_rank=256)
            return {"gA": gA, "gB": gB}

        def tile_a(t, j, first, last):
            b, r = divmod(t, GB)
            if r == 0:
                batch_bufs[b] = gather_batch(b)
                batch_bufs.pop(b - 2, None)
            bb = batch_bufs[b]
            hiT = bb["gA"][:, 0, r * TILE:(r + 1) * TILE]
            hjloT = bb["gA"][:, 0, (GB + r) * TILE:(GB + r + 1) * TILE]
            hjhiT = bb["gB"][:, 0, r * TILE:(r + 1) * TILE]

            d2b = wkp.tile([1, TILE], DT.bfloat16, tag="d2b")
            nc.sync.dma_start(d2b[:], d2_d[:, (b * GB + r) * TILE:
                                           (b * GB + r + 1) * TILE])

            z1 = zps.tile([H, TILE], DT.float32, space="PSUM", tag="z")
            nc.tensor.matmul(z1[:], lhsT=W1a[:], rhs=hiT, start=True, stop=False)
            nc.tensor.matmul(z1[:], lhsT=W1b[:], rhs=hjloT, start=False, stop=False)
            nc.tensor.matmul(z1[:], lhsT=W1b[:], rhs=hjhiT, start=False, stop=False)
            nc.tensor.matmul(z1[:], lhsT=w1c[:], rhs=d2b[:],
                             start=False, stop=True)

            s1T = s1p.tile([H, TILE], DT.bfloat16, tag="s1T")
            nc.scalar.activation(s1T[:], z1[:], AFACT, bias=b1c[:])
            s1_tiles[t] = s1T

            if STAGE < 2:
                return
            sq = wkp.tile([H, TILE], DT.bfloat16, tag="sq")
            nc.vector.tensor_mul(sq[:], s1T[:], s1T[:])

            if first:
                st_t = sps.tile([128, TILE], DT.float32, space="PSUM",
                                tag="stats")
                stats_cur[0] = st_t
            st = stats_cur[0]
            # mu -> st[32:48] (pos (0,32)), E[s1^2] -> st[0:16] (pos (0,0))
            nc.tensor.matmul(st[32:32 + G, :], lhsT=sels[j][:], rhs=s1T[:],
                             start=first, stop=last, skip_group_check=True)
            nc.tensor.matmul(st[0:G, :], lhsT=sels[j][:], rhs=sq[:],
                             start=first, stop=False, skip_group_check=True)

        def ln_group():
            st = stats_cur[0]
            mu2 = stp.tile([32 + G, TILE], DT.bfloat16, tag="mu2")
            nc.scalar.activation(mu2[32:32 + G, :], st[32:32 + G, :], AF.Square)
            # var = E[s1^2] - mu^2 accumulated into st[0:16] (pos (32,0))
            nc.tensor.matmul(st[0:G, :], lhsT=negI[32:32 + G, :],
                             rhs=mu2[32:32 + G, :], start=False, stop=True,
                             skip_group_check=True)
            # rstd = exp(-0.5 * ln(var + eps)) — Log/Exp share one table set
            lnv = stp.tile([G, TILE], DT.float32, tag="lnv")
            nc.scalar.activation(lnv[:], st[0:G, :], AF.Ln, bias=epsc[:])
            rst = stp.tile([G, TILE], DT.bfloat16, tag="rst")
            nc.scalar.activation(rst[:], lnv[:], AF.Exp, scale=-0.5)
            return rst

        def load_bstreams(b):
            nt_b = min(GB, NT - b * GB)
            relb = bsp.tile([128, GB * TPC], DT.bfloat16, tag="relb")
            nc.sync.dma_start(relb[:, :nt_b * TPC],
                              rel_d[:, b * GB * TPC:b * GB * TPC + nt_b * TPC])
            dxb = bsp.tile([128, GB * TPC * 3], DT.float32, tag="dxb")
            nc.sync.dma_start(dxb[:, :nt_b * TPC * 3],
                              dxp_d[:, b * GB * TPC * 3:(b * GB + nt_b) * TPC * 3])
            bstream.update(rel=relb, dx=dxb)

        def tile_b(t, j, rst):
            if STAGE < 3:
                return
            b, r = divmod(t, GB)
            if r == 0:
                load_bstreams(b)
            s1T = s1_tiles.pop(t)

            # rstd scales columns, so it commutes through W2c: scale s1 first
            rb = zps.tile([128, TILE], DT.float32, space="PSUM", tag="z")
            nc.tensor.matmul(rb[:], lhsT=selr[j][:], rhs=rst[:],
                             start=True, stop=True)
            s1n = wkp.tile([H, TILE], DT.bfloat16, tag="s1n")
            nc.vector.tensor_mul(s1n[:], s1T[:], rb[:])
            z2 = zps.tile([H, TILE], DT.float32, space="PSUM", tag="z")
            nc.tensor.matmul(z2[:], lhsT=W2c[:], rhs=s1n[:], start=True, stop=True)

            s2T = wkp.tile([H, TILE], DT.bfloat16, tag="s2T")
            nc.scalar.activation(s2T[:], z2[:], AFACT, bias=b2p[:])

            z3 = zps.tile([H, TILE], DT.float32, space="PSUM", tag="z")
            nc.tensor.matmul(z3[:], lhsT=W3b[:], rhs=s2T[:], start=True, stop=True)
            s3T = wkp.tile([H, TILE], DT.bfloat16, tag="s3T")
            nc.scalar.activation(s3T[:], z3[:], AFACT, bias=b3c[:])

            pp = pps.tile([128, 8], DT.float32, space="PSUM", tag="pp")
            for cc in range(TPC):
                nc.tensor.matmul(pp[:, 4 + cc:5 + cc],
                                 lhsT=s3T[:, cc * CHUNK:(cc + 1) * CHUNK],
                                 rhs=W4b[:], start=True, stop=True,
                                 skip_group_check=True)

            vec = wkp.tile([128, TPC, 3], DT.bfloat16, tag="vec")
            nc.vector.scalar_tensor_tensor(
                out=vec[:],
                in0=pp[:, 4:8][:, :, None].to_broadcast([128, TPC, 3]),
                scalar=b4c[:],
                in1=bstream["dx"][:, r * TPC * 3:(r + 1) * TPC * 3].rearrange(
                    "p (c d) -> p c d", c=TPC),
                op0=ALU.add, op1=ALU.mult)

            if STAGE < 4:
                return
            oht = wkp.tile([128, TPC, WSZ], DT.bfloat16, tag="oht")
            nc.vector.tensor_tensor(
                out=oht[:], in0=iota4[:],
                in1=bstream["rel"][:, r * TPC:(r + 1) * TPC][:, :, None]
                    .to_broadcast([128, TPC, WSZ]),
                op=ALU.is_equal)
            for cc in range(TPC):
                nc.tensor.matmul(pp[:, 0:3], lhsT=oht[:, cc, :],
                                 rhs=vec[:, cc, :],
                                 start=(cc == 0), stop=(cc == TPC - 1),
                                 skip_group_check=True)

            nc.vector.tensor_copy(agg_sb[:, t, 0:3], pp[:, 0:3])

        t0 = 0
        for gi, gsz in enumerate(gsizes if STAGE >= 1 else []):
            for j in range(gsz):
                tile_a(t0 + j, j, j == 0, j == gsz - 1)
            rst = ln_group() if STAGE >= 2 else None
            for j in range(gsz):
                tile_b(t0 + j, j, rst)
            if STAGE < 3:
                s1_tiles.clear()
            t0 += gsz

        # ---- tail: dump per-tile panes; host places them into node rows ----
        nc.sync.dma_start(out_d[:, :],
                          agg_sb[:, :, :].rearrange("p t d -> p (t d)"))

        for _p in reversed(_pools):
            _p.release()

    nc.compile()
    return nc


_CACHE = {}


def _get_nc(sm):
    key = hashlib.sha256(repr(sorted(sm.items())).encode()).hexdigest()
    if key not in _CACHE:
        _CACHE[key] = _build(sm)
    return _CACHE[key]


# ------------------------------------------------------------------- entry --

def kernel(h, x, e, dx, d2, W1, b1, g1, beta1, W2, b2, W3, b3, W4, b4):
    from concourse import bass_utils

    h = np.asarray(h); x = np.asarray(x); e = np.asarray(e)
    dx = np.asarray(dx); d2 = np.asarray(d2)
    data, sm = _prepare(h, x, e, dx, d2)
    nc = _get_nc(sm)

    wmats = {
        "W1": np.asarray(W1, f32), "W2": np.asarray(W2, f32), "W3": np.asarray(W3, f32),
        "W4": np.asarray(W4, f32).reshape(H, 1),
        "b1": np.asarray(b1, f32).reshape(H, 1), "b2": np.asarray(b2, f32).reshape(H, 1),
        "b3": np.asarray(b3, f32).reshape(H, 1),
        "b4": np.full((H, 1), np.asarray(b4, f32).reshape(-1)[0], f32),
        "g1": np.asarray(g1, f32).reshape(H, 1),
        "beta1": np.asarray(beta1, f32).reshape(H, 1),
    }
    sels_h = np.zeros((H, G * G), bf16)
    selr_h = np.zeros((G, G * H), bf16)
    for j in range(G):
        sels_h[:, j * G + j] = bf16(1.0 / H)
        selr_h[j, j * H:(j + 1) * H] = bf16(1.0)
    wmats["sels"] = sels_h
    wmats["selr"] = selr_h
    wmats["negI"] = (-np.eye(G)).astype(bf16)
    in_maps = []
    for c in range(NCORES):
        d = data[c]
        m = {"h_tok": d["h_tok"], "idxA": d["idxA"], "idxB": d["idxB"],
             "rel": d["rel"], "dxp": d["dxp"], "d2": d["d2"]}
        m.update(wmats)
        in_maps.append(m)

    res = bass_utils.run_bass_kernel_spmd(nc, in_maps, core_ids=list(range(NCORES)),
                                          trace=TRACE, tmpdir=TRACE_DIR)
    kernel._last_result = res

    # unshard: place each tile's pane rows into their node rows (each node
    # lives in exactly one tile across all cores)
    NT = sm["NT"]
    out = np.asarray(x, f32).copy()
    for c in range(NCORES):
        d = data[c]
        vals = np.asarray(res.results[c]["out"]).reshape(128, NT, 3)
        rows = d["place_rows"]
        if rows.size:
            out[rows] += vals[d["place_p"], d["place_t"]]
    return out.astype(np.float32)


# revision 36
# speedup vs baseline: 1.0026x; 1.0026x over previous
"""Trainium2 Bass kernel for nn_EquivariantUpdateLayer (GNN message passing).

Edge-parallel across 8 NeuronCores, destination-sorted edge tiles so the
per-node aggregation is local to each core (per sharding hint).

v2 design (vs the HBM-gather baseline):
- The whole h table lives in SBUF as bf16 ([128, 393*128] token layout,
  row n -> partition n%128, rank n//128, per-core ROTATED by the core's
  window base so hi tokens start at rank 1).  Gathers are SBUF->SBUF
  dma_gather (engine cost is per-index descriptor generation).
- 2 gathers per 8-tile batch: A = [hi tokens | hj-lo tokens] (full view),
  B = hj-hi tokens (view offset 255 ranks).  int16 token limit handled by
  the rank-0/rank-392 zero rows as dummy targets; hjT = A_hj + B on DVE.
- LayerNorm: mean correction folded into W2 (W2c = W2g - 1*(1^T W2g)/H),
  so only rstd is needed per edge.  Stats matmuls emit mu / E[s1^2]
  directly (selector value 1/H); var = E[s1^2] - mu^2 via a negated
  identity matmul accumulated into the stats PSUM; rstd = ACT Rsqrt.
  rstd broadcast to [128, e] via a ones-column matmul into PSUM.
- Scale tail: vec = (scp + b4) * dx in one scalar_tensor_tensor; pane
  one-hot built in one batched is_equal over [128, TPC, 128].
- Scatter: per-tile [128,3] panes accumulate in SBUF, dumped once at the
  end; the host places pane rows into node rows (pure permutation) + x.
Host prep does only index manipulation, dtype casts, permutation/padding.
"""
import hashlib
import numpy as np
import ml_dtypes

bf16 = ml_dtypes.bfloat16
f32 = np.float32

NCORES = 8
CHUNK = 128
TPC = 4
TILE = CHUNK * TPC
G = 16             # tiles per LN-stats group
WBITS = 7
WSZ = 128
H = 128
EPS = 1e-5
GB = 8             # tiles per gather/scatter batch
TRACE = False      # set True to capture an NTFF profile (exec_time_ns)
TRACE_DIR = None   # optional fixed dir for NTFF/perfetto artifacts
ACT_FN = "Silu"    # sim_test swaps to Sigmoid (CoreSim lacks Silu)
STAGE = 5          # debug: 1=z1/silu1 2=+stats/LN 3=+z2..scale 4=+scatter 5=full

N_PAD = 50048          # 391*128 (h rows padded)
NRANKS = 391
TOKC = 393 * 128       # table elems/partition incl zero ranks 0 and 392
HIVIEW = 255 * 128     # elem offset of the hj-hi view (rank 255)
LO_MAX = 254 * 128     # rot < LO_MAX uses the lo view (idx = rot+128 < 32640)


# ---------------------------------------------------------------- host prep --

def _pack_tiles(ei_sorted):
    """Pack whole destination nodes into tiles of <=512 edges spanning <128 nodes.
    Returns list of tiles: (edge_start, edge_end, base_node)."""
    nodes, counts = np.unique(ei_sorted, return_counts=True)
    nodes = nodes.tolist()
    counts = counts.tolist()
    tiles = []
    cur_s = 0
    cur_e = 0
    cur_base = -1
    pos = 0
    for node, deg in zip(nodes, counts):
        assert deg <= TILE, f"node degree {deg} > {TILE} unsupported"
        fits = (cur_base >= 0 and (cur_e - cur_s) + deg <= TILE
                and node - cur_base < WSZ)
        if not fits:
            if cur_base >= 0:
                tiles.append((cur_s, cur_e, cur_base))
            cur_s = pos
            cur_e = pos
            cur_base = node
        cur_e += deg
        pos += deg
    if cur_base >= 0:
        tiles.append((cur_s, cur_e, cur_base))
    return tiles


def _wrap16(idx):
    w = idx.reshape(-1, 16).T.astype(np.int16)
    return np.ascontiguousarray(np.tile(w, (8, 1)))


def _prepare(h, x, e, dx, d2):
    N = h.shape[0]
    assert N <= N_PAD
    order = np.argsort(e[0], kind="stable")
    ei = e[0][order].astype(np.int64)
    ej = e[1][order].astype(np.int64)
    dxs = np.asarray(dx, f32)[order]
    d2s = np.asarray(d2, f32)[order][:, 0]

    tiles = _pack_tiles(ei)
    ntiles_tot = len(tiles)
    NT = -(-ntiles_tot // NCORES)
    ngroups = -(-NT // G)
    gsizes = tuple(min(G, NT - g * G) for g in range(ngroups))
    NB = -(-NT // GB)

    cores = []
    for c in range(NCORES):
        lo = min(c * NT, ntiles_tot)
        hi = min(lo + NT, ntiles_tot)
        ct = tiles[lo:hi]
        if ct:
            wfirst = ct[0][2] >> WBITS
            wlast = (ct[-1][2] + WSZ - 1) >> WBITS
        else:
            wfirst, wlast = 0, 0
        cores.append({"tiles": ct, "wfirst": wfirst,
                      "nwin": max(wlast - wfirst + 1, 1)})
    nwin_cap = max(cr["nwin"] for cr in cores)
    nwin_cap = -(-nwin_cap // 8) * 8
    R = nwin_cap * WSZ + WSZ  # + dump zone; dump row = R-1
    assert R <= 32767
    assert nwin_cap * WSZ + 256 < LO_MAX, "window exceeds lo-view token range"

    C = NT * TILE
    CB = NB * GB * TILE  # batch-padded edge capacity
    h_pad = np.zeros((N_PAD, H), f32)
    h_pad[:N] = np.asarray(h, f32)
    h_pad_b = h_pad.astype(bf16)

    # ownership boundaries for the host combine
    bnds = []
    prev = 0
    for cc in range(NCORES):
        if cores[cc]["tiles"]:
            prev = cores[cc]["tiles"][-1][2] + WSZ
        bnds.append(prev)

    data = []
    for c in range(NCORES):
        cr = cores[c]
        base0 = cr["wfirst"] << WBITS
        ei_f = np.full(CB, base0, np.int64)
        ej_f = np.zeros(CB, np.int64)
        dx_f = np.zeros((CB, 3), f32)
        d2_f = np.zeros(CB, f32)
        rel_f = np.zeros(CB, f32)
        tbase = np.full(NT, base0, np.int64)
        tspan = np.zeros(NT, np.int64)
        for k, (s, t, tb) in enumerate(cr["tiles"]):
            n = t - s
            sl = slice(k * TILE, k * TILE + n)
            ei_f[sl] = ei[s:t]
            ej_f[sl] = ej[s:t]
            dx_f[sl] = dxs[s:t]
            d2_f[sl] = d2s[s:t]
            rel_f[sl] = (ei[s:t] - tb).astype(f32)
            ei_f[k * TILE + n:(k + 1) * TILE] = tb
            tbase[k] = tb
            tspan[k] = int(ei[t - 1] - tb + 1) if n else 0

        # gather tokens (rotated table: rank 1+k holds rows base0+128k..)
        hi_tok = ei_f - base0 + 128
        rot = (ej_f - base0) % N_PAD
        in_lo = rot < LO_MAX
        # lo view: idx = rot+128 in [128, 32640); dummy = rank 0 (zeros)
        # hi view (offset 255 ranks): abs rank 255 + t//128 = 1 + rot//128
        #   -> t = rot - 254*128 = rot - 32512; dummy = view rank 137 (zeros)
        tokA_hj = np.where(in_lo, rot + 128, ej_f % 128)
        tokB = np.where(in_lo, 137 * 128 + (ej_f % 128), rot - 32512)
        assert tokB.max() < 138 * 128 and tokB.min() >= 0
        assert tokA_hj.max() < 32640 and hi_tok.max() < 32640

        # per-batch idx arrays: A = [hi | hj-lo] (8192), B = hj-hi (4096)
        idxA = np.zeros((NB, 2 * GB * TILE), np.int64)
        idxB = np.zeros((NB, GB * TILE), np.int64)
        for b in range(NB):
            sl = slice(b * GB * TILE, (b + 1) * GB * TILE)
            idxA[b, :GB * TILE] = hi_tok[sl]
            idxA[b, GB * TILE:] = tokA_hj[sl]
            idxB[b] = tokB[sl]
        idxA_w = _wrap16(idxA.reshape(-1))
        idxB_w = _wrap16(idxB.reshape(-1))

        # host pane-placement indices: node row <- (pane partition p, tile t)
        rows_l, p_l, t_l = [], [], []
        for k in range(NT):
            sp = int(tspan[k])
            if sp > 0:
                rows_l.append(tbase[k] + np.arange(sp))
                p_l.append(np.arange(sp))
                t_l.append(np.full(sp, k))
        place_rows = np.concatenate(rows_l) if rows_l else np.zeros(0, np.int64)
        place_p = np.concatenate(p_l) if p_l else np.zeros(0, np.int64)
        place_t = np.concatenate(t_l) if t_l else np.zeros(0, np.int64)

        rel_p = rel_f.reshape(NB * GB, TPC, CHUNK).transpose(0, 2, 1)
        rel_packed = rel_p.transpose(1, 0, 2).reshape(CHUNK, NB * GB * TPC)
        dx_p = dx_f.reshape(NB * GB, TPC, CHUNK, 3).transpose(0, 2, 1, 3)
        dx_packed = dx_p.transpose(1, 0, 2, 3).reshape(CHUNK, NB * GB * TPC * 3)

        # rotated token table [128, TOKC]: rank0/rank392 zeros
        q = np.arange(NRANKS)
        p = np.arange(128)
        rows_idx = (base0 + q[None, :] * 128 + p[:, None]) % N_PAD  # [128, 391]
        tbl = h_pad_b[rows_idx]                                     # [128,391,H]
        h_tok = np.zeros((128, TOKC), bf16)
        h_tok[:, 128:128 + NRANKS * 128] = tbl.reshape(128, NRANKS * 128)

        data.append({
            "h_tok": np.ascontiguousarray(h_tok),
            "idxA": idxA_w, "idxB": idxB_w,
            "rel": np.ascontiguousarray(rel_packed.astype(bf16)),
            "dxp": np.ascontiguousarray(dx_packed, dtype=f32),
            "d2": np.ascontiguousarray(d2_f.astype(bf16)[None, :]),
            "place_rows": place_rows, "place_p": place_p, "place_t": place_t,
            "wfirst": cr["wfirst"],
        })

    shape_meta = {"C": C, "NT": NT, "NB": NB, "gsizes": gsizes,
                  "nwin_cap": nwin_cap, "R": R, "N": N}
    return data, shape_meta


# ------------------------------------------------------------- graph builder --

def _build(sm):
    import concourse.bass as bass
    import concourse.bacc as bacc
    import concourse.mybir as mybir
    import concourse.tile as tile

    NT, NB, nwin_cap, R = sm["NT"], sm["NB"], sm["nwin_cap"], sm["R"]
    gsizes = sm["gsizes"]
    AF = mybir.ActivationFunctionType
    AFACT = getattr(AF, ACT_FN)
    DT = mybir.dt
    ALU = mybir.AluOpType

    nc = bacc.Bacc("TRN2", num_devices=NCORES)

    def din(name, shape, dt):
        return nc.dram_tensor(name, shape, dt, kind="ExternalInput").ap()

    h_tok_d = din("h_tok", [128, TOKC], DT.bfloat16)
    idxA_d = din("idxA", [128, NB * 2 * GB * TILE // 16], DT.int16)
    idxB_d = din("idxB", [128, NB * GB * TILE // 16], DT.int16)
    rel_d = din("rel", [128, NB * GB * TPC], DT.bfloat16)
    dxp_d = din("dxp", [128, NB * GB * TPC * 3], DT.float32)
    d2_d = din("d2", [1, NB * GB * TILE], DT.bfloat16)
    sels_d = din("sels", [H, G * G], DT.bfloat16)
    selr_d = din("selr", [G, G * H], DT.bfloat16)
    negI_d = din("negI", [G, G], DT.bfloat16)
    W1_d = din("W1", [2 * H + 1, H], DT.float32)
    W2_d = din("W2", [H, H], DT.float32)
    W3_d = din("W3", [H, H], DT.float32)
    W4_d = din("W4", [H, 1], DT.float32)
    b1_d = din("b1", [H, 1], DT.float32)
    b2_d = din("b2", [H, 1], DT.float32)
    b3_d = din("b3", [H, 1], DT.float32)
    b4_d = din("b4", [H, 1], DT.float32)
    g1_d = din("g1", [H, 1], DT.float32)
    beta_d = din("beta1", [H, 1], DT.float32)
    out_d = nc.dram_tensor("out", [128, NT * 3], DT.float32,
                           kind="ExternalOutput").ap()

    with tile.TileContext(nc) as tc:
        _pools = []

        def _mkpool(**kw):
            p = tc.alloc_tile_pool(**kw)
            _pools.append(p)
            return p

        con = _mkpool(name="con", bufs=1)
        zps = _mkpool(name="zps", bufs=4, space="PSUM")
        sps = _mkpool(name="sps", bufs=1, space="PSUM")
        pps = _mkpool(name="pps", bufs=2, space="PSUM")
        gAp = _mkpool(name="gAp", bufs=2)
        gBp = _mkpool(name="gBp", bufs=2)
        gip = _mkpool(name="gip", bufs=2)
        s1p = _mkpool(name="s1p", bufs=G + 4)
        wkp = _mkpool(name="wkp", bufs=2)
        stp = _mkpool(name="stp", bufs=2)
        bsp = _mkpool(name="bsp", bufs=2)

        # ---- one-time constants ----
        def load_cast(dram_ap, shape, name):
            t_f = con.tile(shape, DT.float32, tag=f"{name}_f")
            nc.sync.dma_start(t_f[:], dram_ap)
            t_b = con.tile(shape, DT.bfloat16, tag=name)
            nc.vector.tensor_copy(t_b[:], t_f[:])
            return t_b

        W1a = load_cast(W1_d[0:H, :], [H, H], "W1a")
        W1b = load_cast(W1_d[H:2 * H, :], [H, H], "W1b")
        w1c = load_cast(W1_d[2 * H:2 * H + 1, :], [1, H], "w1c")
        W3b = load_cast(W3_d[:, :], [H, H], "W3b")
        W4b = load_cast(W4_d[:, :], [H, 1], "W4b")
        W2b = load_cast(W2_d[:, :], [H, H], "W2b")
        betab = load_cast(beta_d[:, :], [H, 1], "betab")

        def load_col(dram_ap, name):
            t = con.tile([H, 1], DT.float32, tag=name)
            nc.sync.dma_start(t[:], dram_ap)
            return t

        b1c = load_col(b1_d[:, :], "b1c")
        b2c = load_col(b2_d[:, :], "b2c")
        b3c = load_col(b3_d[:, :], "b3c")
        b4c = load_col(b4_d[:, :], "b4c")
        g1c = load_col(g1_d[:, :], "g1c")
        W2f = con.tile([H, H], DT.float32, tag="W2f")
        nc.sync.dma_start(W2f[:], W2_d[:, :])

        W2gf = con.tile([H, H], DT.float32, tag="W2gf")
        nc.vector.tensor_scalar_mul(W2gf[:], W2f[:], g1c[:])
        W2g = con.tile([H, H], DT.bfloat16, tag="W2g")
        nc.vector.tensor_copy(W2g[:], W2gf[:])

        onesc = con.tile([H, 1], DT.bfloat16, tag="onesc")
        nc.vector.memset(onesc[:], 1.0)
        ones1 = con.tile([1, 1], DT.bfloat16, tag="ones1")
        nc.vector.memset(ones1[:], 1.0)
        onesr = con.tile([1, H], DT.bfloat16, tag="onesr")
        nc.vector.memset(onesr[:], 1.0)

        # W2c = W2g - ones (x) (1^T W2g)/H   (folds -u*mu into W2)
        u_ps = zps.tile([1, H], DT.float32, space="PSUM", tag="z")
        nc.tensor.matmul(u_ps[:], lhsT=onesc[:], rhs=W2g[:], start=True, stop=True)
        u_row = con.tile([1, H], DT.bfloat16, tag="u_row")
        nc.vector.tensor_scalar_mul(u_row[:], u_ps[:], 1.0 / H)
        rk_ps = zps.tile([H, H], DT.float32, space="PSUM", tag="z")
        nc.tensor.matmul(rk_ps[:], lhsT=onesr[:], rhs=u_row[:], start=True, stop=True)
        W2c = con.tile([H, H], DT.bfloat16, tag="W2c")
        nc.vector.tensor_sub(W2c[:], W2gf[:], rk_ps[:])

        # b2p = W2^T beta + b2
        bb_ps = zps.tile([1, H], DT.float32, space="PSUM", tag="z")
        nc.tensor.matmul(bb_ps[:], lhsT=betab[:], rhs=W2b[:], start=True, stop=True)
        bb_row = con.tile([1, H], DT.bfloat16, tag="bb_row")
        nc.vector.tensor_copy(bb_row[:], bb_ps[:])
        bbT_ps = zps.tile([H, 1], DT.float32, space="PSUM", tag="z")
        nc.tensor.matmul(bbT_ps[:], lhsT=bb_row[:], rhs=ones1[:], start=True, stop=True)
        b2p = con.tile([H, 1], DT.float32, tag="b2p")
        nc.vector.tensor_add(b2p[:], bbT_ps[:], b2c[:])

        # iota4 [128, TPC, WSZ] bf16 (value = col within chunk, all partitions)
        iotai = con.tile([128, TPC, WSZ], DT.int32, tag="iotai")
        nc.gpsimd.iota(iotai[:], pattern=[[0, TPC], [1, WSZ]], base=0,
                       channel_multiplier=0)
        iota4 = con.tile([128, TPC, WSZ], DT.bfloat16, tag="iota4")
        nc.vector.tensor_copy(iota4[:], iotai[:])

        # stats selectors (value 1/H), row selectors for the rstd broadcast,
        # and a negated identity living at partitions 32:48 (so the var
        # matmul's operand bases match its tile_position row group).
        # Host-provided (per-row writes would need unaligned partition APs).
        sels_t = con.tile([H, G * G], DT.bfloat16, tag="sels_t")
        nc.sync.dma_start(sels_t[:], sels_d[:, :])
        sels = [sels_t[:, j * G:(j + 1) * G] for j in range(G)]
        selr_t = con.tile([G, G * H], DT.bfloat16, tag="selr_t")
        nc.sync.dma_start(selr_t[:], selr_d[:, :])
        selr = [selr_t[:, j * H:(j + 1) * H] for j in range(G)]
        negI = con.tile([32 + G, G], DT.bfloat16, tag="negI")
        nc.sync.dma_start(negI[32:32 + G, :], negI_d[:, :])
        epsc = con.tile([G, 1], DT.float32, tag="epsc")
        nc.vector.memset(epsc[:], EPS)

        # resident token table + per-tile pane accumulator
        ht = con.tile([128, TOKC], DT.bfloat16, tag="ht")
        nc.sync.dma_start(ht[:], h_tok_d[:, :])
        agg_sb = con.tile([128, NT, 3], DT.float32, tag="agg_sb")

        # ---- pipeline ----
        batch_bufs = {}
        s1_tiles = {}
        stats_cur = [None]
        bstream = {}

        def gather_batch(b):
            iA = gip.tile([128, 2 * GB * TILE // 16], DT.int16, tag="iA")
            nc.sync.dma_start(iA[:], idxA_d[:, b * (2 * GB * TILE // 16):
                                            (b + 1) * (2 * GB * TILE // 16)])
            iB = gip.tile([128, GB * TILE // 16], DT.int16, tag="iB")
            nc.sync.dma_start(iB[:], idxB_d[:, b * (GB * TILE // 16):
                                            (b + 1) * (GB * TILE // 16)])
            gA = gAp.tile([128, 1, 2 * GB * TILE], DT.bfloat16, tag="gA")
            gB = gBp.tile([128, 1, GB * TILE], DT.bfloat16, tag="gB")
            nc.gpsimd.dma_gather(
                out_ap=gA[:, :, :], in_ap=ht[:, :], idxs_ap=iA[:, :],
                num_idxs=2 * GB * TILE, num_idxs_reg=2 * GB * TILE,
                elem_size=H, transpose=True, single_packet=False,
                sbuf_tokens_per_rank=128, sbuf_free_dim_per_rank=256)
            nc.gpsimd.dma_gather(
                out_ap=gB[:, :, :], in_ap=ht[:, HIVIEW:], idxs_ap=iB[:, :],
                num_idxs=GB * TILE, num_idxs_reg=GB * TILE,
                elem_size=H, transpose=True, single_packet=False,
                sbuf_tokens_per_rank=128, sbuf_free_dim_per_rank=256)
            return {"gA": gA, "gB": gB}

        def tile_a(t, j, first, last):
            b, r = divmod(t, GB)
            if r == 0:
                batch_bufs[b] = gather_batch(b)
                batch_bufs.pop(b - 2, None)
            bb = batch_bufs[b]
            hiT = bb["gA"][:, 0, r * TILE:(r + 1) * TILE]
            hjloT = bb["gA"][:, 0, (GB + r) * TILE:(GB + r + 1) * TILE]
            hjhiT = bb["gB"][:, 0, r * TILE:(r + 1) * TILE]

            d2b = wkp.tile([1, TILE], DT.bfloat16, tag="d2b")
            nc.sync.dma_start(d2b[:], d2_d[:, (b * GB + r) * TILE:
                                           (b * GB + r + 1) * TILE])

            z1 = zps.tile([H, TILE], DT.float32, space="PSUM", tag="z")
            nc.tensor.matmul(z1[:], lhsT=W1a[:], rhs=hiT, start=True, stop=False)
            nc.tensor.matmul(z1[:], lhsT=W1b[:], rhs=hjloT, start=False, stop=False)
            nc.tensor.matmul(z1[:], lhsT=W1b[:], rhs=hjhiT, start=False, stop=False)
            nc.tensor.matmul(z1[:], lhsT=w1c[:], rhs=d2b[:],
                             start=False, stop=True)

            s1T = s1p.tile([H, TILE], DT.bfloat16, tag="s1T")
            nc.scalar.activation(s1T[:], z1[:], AFACT, bias=b1c[:])
            s1_tiles[t] = s1T

            if STAGE < 2:
                return
            sq = wkp.tile([H, TILE], DT.bfloat16, tag="sq")
            nc.vector.tensor_mul(sq[:], s1T[:], s1T[:])

            if first:
                st_t = sps.tile([128, TILE], DT.float32, space="PSUM",
                                tag="stats")
                stats_cur[0] = st_t
            st = stats_cur[0]
            # mu -> st[32:48] (pos (0,32)), E[s1^2] -> st[0:16] (pos (0,0))
            nc.tensor.matmul(st[32:32 + G, :], lhsT=sels[j][:], rhs=s1T[:],
                             start=first, stop=last, skip_group_check=True)
            nc.tensor.matmul(st[0:G, :], lhsT=sels[j][:], rhs=sq[:],
                             start=first, stop=False, skip_group_check=True)

        def ln_group():
            st = stats_cur[0]
            mu2 = stp.tile([32 + G, TILE], DT.bfloat16, tag="mu2")
            nc.scalar.activation(mu2[32:32 + G, :], st[32:32 + G, :], AF.Square)
            # var = E[s1^2] - mu^2 accumulated into st[0:16] (pos (32,0))
            nc.tensor.matmul(st[0:G, :], lhsT=negI[32:32 + G, :],
                             rhs=mu2[32:32 + G, :], start=False, stop=True,
                             skip_group_check=True)
            # rstd = exp(-0.5 * ln(var + eps)) — Log/Exp share one table set
            lnv = stp.tile([G, TILE], DT.float32, tag="lnv")
            nc.scalar.activation(lnv[:], st[0:G, :], AF.Ln, bias=epsc[:])
            rst = stp.tile([G, TILE], DT.bfloat16, tag="rst")
            nc.scalar.activation(rst[:], lnv[:], AF.Exp, scale=-0.5)
            return rst

        def load_bstreams(b):
            nt_b = min(GB, NT - b * GB)
            relb = bsp.tile([128, GB * TPC], DT.bfloat16, tag="relb")
            nc.sync.dma_start(relb[:, :nt_b * TPC],
                              rel_d[:, b * GB * TPC:b * GB * TPC + nt_b * TPC])
            dxb = bsp.tile([128, GB * TPC * 3], DT.float32, tag="dxb")
            nc.sync.dma_start(dxb[:, :nt_b * TPC * 3],
                              dxp_d[:, b * GB * TPC * 3:(b * GB + nt_b) * TPC * 3])
            bstream.update(rel=relb, dx=dxb)

        def tile_b(t, j, rst):
            if STAGE < 3:
                return
            b, r = divmod(t, GB)
            if r == 0:
                load_bstreams(b)
            s1T = s1_tiles.pop(t)

            # rstd scales columns, so it commutes through W2c: scale s1 first
            rb = zps.tile([128, TILE], DT.float32, space="PSUM", tag="z")
            nc.tensor.matmul(rb[:], lhsT=selr[j][:], rhs=rst[:],
                             start=True, stop=True)
            s1n = wkp.tile([H, TILE], DT.bfloat16, tag="s1n")
            nc.vector.tensor_mul(s1n[:], s1T[:], rb[:])
            z2 = zps.tile([H, TILE], DT.float32, space="PSUM", tag="z")
            nc.tensor.matmul(z2[:], lhsT=W2c[:], rhs=s1n[:], start=True, stop=True)

            s2T = wkp.tile([H, TILE], DT.bfloat16, tag="s2T")
            nc.scalar.activation(s2T[:], z2[:], AFACT, bias=b2p[:])

            z3 = zps.tile([H, TILE], DT.float32, space="PSUM", tag="z")
            nc.tensor.matmul(z3[:], lhsT=W3b[:], rhs=s2T[:], start=True, stop=True)
            s3T = wkp.tile([H, TILE], DT.bfloat16, tag="s3T")
            nc.scalar.activation(s3T[:], z3[:], AFACT, bias=b3c[:])

            pp = pps.tile([128, 8], DT.float32, space="PSUM", tag="pp")
            for cc in range(TPC):
                nc.tensor.matmul(pp[:, 4 + cc:5 + cc],
                                 lhsT=s3T[:, cc * CHUNK:(cc + 1) * CHUNK],
                                 rhs=W4b[:], start=True, stop=True,
                                 skip_group_check=True)

            vec = wkp.tile([128, TPC, 3], DT.bfloat16, tag="vec")
            nc.vector.scalar_tensor_tensor(
                out=vec[:],
                in0=pp[:, 4:8][:, :, None].to_broadcast([128, TPC, 3]),
                scalar=b4c[:],
                in1=bstream["dx"][:, r * TPC * 3:(r + 1) * TPC * 3].rearrange(
                    "p (c d) -> p c d", c=TPC),
                op0=ALU.add, op1=ALU.mult)

            if STAGE < 4:
                return
            oht = wkp.tile([128, TPC, WSZ], DT.bfloat16, tag="oht")
            nc.vector.tensor_tensor(
                out=oht[:], in0=iota4[:],
                in1=bstream["rel"][:, r * TPC:(r + 1) * TPC][:, :, None]
                    .to_broadcast([128, TPC, WSZ]),
                op=ALU.is_equal)
            for cc in range(TPC):
                nc.tensor.matmul(pp[:, 0:3], lhsT=oht[:, cc, :],
                                 rhs=vec[:, cc, :],
                                 start=(cc == 0), stop=(cc == TPC - 1),
                                 skip_group_check=True)

            nc.vector.tensor_copy(agg_sb[:, t, 0:3], pp[:, 0:3])

        t0 = 0
        for gi, gsz in enumerate(gsizes if STAGE >= 1 else []):
            for j in range(gsz):
                tile_a(t0 + j, j, j == 0, j == gsz - 1)
            rst = ln_group() if STAGE >= 2 else None
            for j in range(gsz):
                tile_b(t0 + j, j, rst)
            if STAGE < 3:
                s1_tiles.clear()
            t0 += gsz

        # ---- tail: dump per-tile panes; host places them into node rows ----
        nc.sync.dma_start(out_d[:, :],
                          agg_sb[:, :, :].rearrange("p t d -> p (t d)"))

        for _p in reversed(_pools):
            _p.release()

    nc.compile()
    return nc


_CACHE = {}


def _get_nc(sm):
    key = hashlib.sha256(repr(sorted(sm.items())).encode()).hexdigest()
    if key not in _CACHE:
        _CACHE[key] = _build(sm)
    return _CACHE[key]


# ------------------------------------------------------------------- entry --

def kernel(h, x, e, dx, d2, W1, b1, g1, beta1, W2, b2, W3, b3, W4, b4):
    from concourse import bass_utils

    h = np.asarray(h); x = np.asarray(x); e = np.asarray(e)
    dx = np.asarray(dx); d2 = np.asarray(d2)
    data, sm = _prepare(h, x, e, dx, d2)
    nc = _get_nc(sm)

    wmats = {
        "W1": np.asarray(W1, f32), "W2": np.asarray(W2, f32), "W3": np.asarray(W3, f32),
        "W4": np.asarray(W4, f32).reshape(H, 1),
        "b1": np.asarray(b1, f32).reshape(H, 1), "b2": np.asarray(b2, f32).reshape(H, 1),
        "b3": np.asarray(b3, f32).reshape(H, 1),
        "b4": np.full((H, 1), np.asarray(b4, f32).reshape(-1)[0], f32),
        "g1": np.asarray(g1, f32).reshape(H, 1),
        "beta1": np.asarray(beta1, f32).reshape(H, 1),
    }
    sels_h = np.zeros((H, G * G), bf16)
    selr_h = np.zeros((G, G * H), bf16)
    for j in range(G):
        sels_h[:, j * G + j] = bf16(1.0 / H)
        selr_h[j, j * H:(j + 1) * H] = bf16(1.0)
    wmats["sels"] = sels_h
    wmats["selr"] = selr_h
    wmats["negI"] = (-np.eye(G)).astype(bf16)
    in_maps = []
    for c in range(NCORES):
        d = data[c]
        m = {"h_tok": d["h_tok"], "idxA": d["idxA"], "idxB": d["idxB"],
             "rel": d["rel"], "dxp": d["dxp"], "d2": d["d2"]}
        m.update(wmats)
        in_maps.append(m)

    res = bass_utils.run_bass_kernel_spmd(nc, in_maps, core_ids=list(range(NCORES)),
                                          trace=TRACE, tmpdir=TRACE_DIR)
    kernel._last_result = res

    # unshard: place each tile's pane rows into their node rows (each node
    # lives in exactly one tile across all cores)
    NT = sm["NT"]
    out = np.asarray(x, f32).copy()
    for c in range(NCORES):
        d = data[c]
        vals = np.asarray(res.results[c]["out"]).reshape(128, NT, 3)
        rows = d["place_rows"]
        if rows.size:
            out[rows] += vals[d["place_p"], d["place_t"]]
    return out.astype(np.float32)
